# revision 16
# baseline (speedup 1.0000x reference)
"""Single-head causal attention (B=4, S=2048, M=H=1024) on 8 Trainium2 cores.

Sharding: core = (batch, half). Core half c owns global q-blocks
{c, c+2, ..., c+14} (stride-2 interleave balances the causal triangle);
local block l <-> global q-block 2l+c, local query cols [128l, 128l+128).

Zero-QK-bias fast path (_build_fast): scores = q @ (Wq.T@Wk) @ k.T with the
weight product A folded on host, so K needs no on-device projection. All
matmul inputs are bf16 (enables the PE's automatic fast-weight-load; fp32r
gets none), PSUM accumulation stays fp32.

Per-core pipeline:
  qh[m',sq] = A.T @ qT      (128 MMs of N=512)
  vh[sk,h]  = vT.T @ WvT    (256 MMs of N=512)
  scores mega-group: slot i = key-block i covers only local q-blocks l with
    2l+c >= i, a suffix of width W_i = 128*(8 - i//2) (SPMD-identical for
    both halves; the boundary 256 cols get a data-driven mask: causal tri /
    zero / ones depending on (c, i parity)).
    scoresT[sk, sq-suffix] = ktr.T @ qh, exp on ACT -> bf16 e tiles
  esum[sk, sq] += e_i       (DVE bf16 adds; softmax denominator feed)
  AV per local block l: out[sq,h] = sum_kb e^T @ vh (N=512 MMs), denominator
    via one tiny matmul lhsT=esum-slice rhs=ones -> [sq, 2]; normalize on
    DVE/ACT halves, DMA out.
"""

import contextlib
import os

import numpy as np

B, S, MD, HD = 4, 2048, 1024, 1024
P = 128
NB = S // P            # 16 key/query blocks per batch
SQL = S // 2           # 1024 local queries per core
N_CORES = 8
MC = MD // P           # 8 contraction chunks
HB = HD // P           # 8 h-blocks


def _build_fast(use_pad: bool, use_vbias: bool):
    import concourse.bacc as bacc
    import concourse.mybir as mybir
    import concourse.tile as tile

    f32 = mybir.dt.float32
    bf16 = mybir.dt.bfloat16
    Act = mybir.ActivationFunctionType

    nc = bacc.Bacc("TRN2", num_swdge_queues=4, dynamic_dma_scratch_size=2048)

    # All inputs are pre-arranged on host into SBUF layout [partition, ...]
    # so every DMA is one contiguous 2-32KB descriptor per partition (the
    # DMA queues are descriptor-rate-bound at ~25-40ns/descriptor).
    qt = nc.dram_tensor("qt", [P, MC, SQL], bf16, kind="ExternalInput")
    kt = nc.dram_tensor("kt", [P, MC, S], bf16, kind="ExternalInput")
    vt = nc.dram_tensor("vt", [P, MC, S], bf16, kind="ExternalInput")
    at = nc.dram_tensor("at", [P, MC, MD], bf16, kind="ExternalInput")
    wvt = nc.dram_tensor("wvt", [P, MC, HD], bf16, kind="ExternalInput")
    masks = nc.dram_tensor("masks", [P, NB, P], bf16, kind="ExternalInput")
    if use_pad:
        padm = nc.dram_tensor("padm", [P, NB], f32, kind="ExternalInput")
    if use_vbias:
        bv = nc.dram_tensor("bv", [HD], bf16, kind="ExternalInput")
    out = nc.dram_tensor("out", [SQL, HD], f32, kind="ExternalOutput")

    with tile.TileContext(nc) as tc:
        with (
            tc.tile_pool(name="res", bufs=1) as res,
            tc.tile_pool(name="w", bufs=1) as wpool,
            tc.tile_pool(name="exp", bufs=24) as epool,
            tc.tile_pool(name="outp", bufs=2) as outp,
            tc.tile_pool(name="small", bufs=2) as small,
            tc.tile_pool(name="mm", bufs=6, space="PSUM") as mmp,
            tc.tile_pool(name="dn", bufs=2, space="PSUM") as dnp,
        ):
            ktr = res.tile([P, MC, S], bf16, tag="ktr")
            qh = res.tile([P, MC, SQL], bf16, tag="qh")
            vh = res.tile([P, NB, HD], bf16, tag="vh")
            mt = res.tile([P, NB, P], bf16, tag="mt")
            esum = res.tile([P, SQL], bf16, tag="esum")
            ones = res.tile([P, 2], bf16, tag="ones")
            nc.vector.memset(ones[:], 1.0)
            nc.vector.memset(esum[:], 0.0)

            # issue DMAs strictly in first-use order; spread queues
            a_t = wpool.tile([P, MC, MD], bf16, tag="a")
            xq = wpool.tile([P, MC, SQL], bf16, tag="xq")
            xv = wpool.tile([P, MC, S], bf16, tag="xv")
            wv_t = wpool.tile([P, MC, HD], bf16, tag="wv")
            nc.sync.dma_start(a_t[:, 0:1, :], at.ap()[:, 0:1, :])
            nc.gpsimd.dma_start(xq[:, 0:1, :], qt.ap()[:, 0:1, :])
            nc.sync.dma_start(a_t[:, 1:8, :], at.ap()[:, 1:8, :])
            nc.gpsimd.dma_start(xq[:, 1:8, :], qt.ap()[:, 1:8, :])
            nc.scalar.dma_start(xv[:, 0:4, :], vt.ap()[:, 0:4, :])
            nc.scalar.dma_start(xv[:, 4:8, :], vt.ap()[:, 4:8, :])
            if use_pad:
                pad_t = res.tile([P, NB], f32, tag="pad")
            if use_vbias:
                ones_row = res.tile([1, P], bf16, tag="or")
                bvr = res.tile([1, HD], bf16, tag="bvr")
                nc.gpsimd.memset(ones_row[:], 1.0)
                nc.gpsimd.dma_start(bvr[:], bv.ap()[None, :])

            # ---- Q rotation: qh[:, mc', sq] = A.T @ qT ----
            for sqc in range(2):
                for hb in range(HB):
                    ps = mmp.tile([P, 512], f32, tag="mm", name=f"q{sqc}_{hb}")
                    for mc in range(MC):
                        nc.tensor.matmul(
                            ps[:], a_t[:, mc, hb * P:(hb + 1) * P],
                            xq[:, mc, sqc * 512:(sqc + 1) * 512],
                            start=(mc == 0), stop=(mc == MC - 1))
                    if hb % 2 == 0:
                        nc.vector.tensor_copy(
                            qh[:, hb, sqc * 512:(sqc + 1) * 512], ps[:])
                    else:
                        nc.scalar.copy(qh[:, hb, sqc * 512:(sqc + 1) * 512],
                                       ps[:])
                if sqc == 0:
                    # later inputs, in need order: wv (~35us), kT (~95us)
                    nc.sync.dma_start(wv_t[:, 0:4, :], wvt.ap()[:, 0:4, :])
                    nc.gpsimd.dma_start(wv_t[:, 4:8, :], wvt.ap()[:, 4:8, :])

            nc.sync.dma_start(ktr[:, 0:4, :], kt.ap()[:, 0:4, :])
            nc.gpsimd.dma_start(ktr[:, 4:8, :], kt.ap()[:, 4:8, :])
            nc.gpsimd.dma_start(mt[:], masks.ap())
            if use_pad:
                nc.gpsimd.dma_start(pad_t[:], padm.ap())

            # ---- V projection: vh[sk, h] (keys on partitions) ----
            for skb in range(NB):
                for hc in range(2):
                    ps = mmp.tile([P, 512], f32, tag="mm",
                                  name=f"v{skb % 3}_{hc}")
                    for mc in range(MC):
                        nc.tensor.matmul(
                            ps[:], xv[:, mc, skb * P:(skb + 1) * P],
                            wv_t[:, mc, hc * 512:(hc + 1) * 512],
                            start=(mc == 0),
                            stop=(mc == MC - 1) and not use_vbias)
                    if use_vbias:
                        nc.tensor.matmul(
                            ps[:], ones_row[:],
                            bvr[:, hc * 512:(hc + 1) * 512],
                            start=False, stop=True)
                    if (skb + hc) % 2 == 0:
                        nc.vector.tensor_copy(
                            vh[:, skb, hc * 512:(hc + 1) * 512], ps[:])
                    else:
                        nc.scalar.copy(
                            vh[:, skb, hc * 512:(hc + 1) * 512], ps[:])

            # ---- scores mega-group: slot i = key-block i ----
            # covers local q cols [128*(i//2), 1024), in 512-col chunks
            etiles = {}
            for i in range(NB):
                s0 = (i // 2) * P
                Wi = SQL - s0
                ncc = (Wi + 511) // 512
                for cc in range(ncc):
                    cw = min(512, Wi - cc * 512)
                    ps = mmp.tile([P, 512], f32, tag="mm", name=f"s{(2*i+cc) % 3}")
                    for mc in range(MC):
                        nc.tensor.matmul(
                            ps[:, 0:cw], ktr[:, mc, i * P:(i + 1) * P],
                            qh[:, mc, s0 + cc * 512: s0 + cc * 512 + cw],
                            start=(mc == 0), stop=(mc == MC - 1))
                    ex = epool.tile([P, 512], bf16, tag="e", name=f"e{i}_{cc}")
                    nc.scalar.activation(ex[:, 0:cw], ps[:, 0:cw], Act.Exp,
                                         scale=1.0 / 32.0)
                    if cc == 0:
                        # only the first 128 cols (this slot's diagonal /
                        # invalid q-block) ever need masking
                        nc.vector.tensor_mul(ex[:, 0:P], ex[:, 0:P],
                                             mt[:, i, :])
                    if use_pad:
                        nc.vector.tensor_scalar_mul(
                            ex[:, 0:cw], ex[:, 0:cw], pad_t[:, i:i + 1])
                    nc.vector.tensor_add(
                        esum[:, s0 + cc * 512: s0 + cc * 512 + cw],
                        esum[:, s0 + cc * 512: s0 + cc * 512 + cw],
                        ex[:, 0:cw])
                    etiles[(i, cc)] = ex

            # ---- AV + denominator + normalize, per local q-block l ----
            def eslice(l, kb):
                pos = (l - kb // 2) * P
                ex = etiles[(kb, pos // 512)]
                off = pos % 512
                return ex[:, off:off + P]

            for l in range(8):
                nkb = 2 * l + 2
                avs = [mmp.tile([P, 512], f32, tag="mm", name=f"av{l % 2}_{hc}")
                       for hc in range(2)]
                for kb in range(nkb):
                    lhs = eslice(l, kb)
                    for hc in range(2):
                        nc.tensor.matmul(
                            avs[hc][:], lhs, vh[:, kb, hc * 512:(hc + 1) * 512],
                            start=(kb == 0), stop=(kb == nkb - 1))
                dps = dnp.tile([P, 2], f32, tag="d")
                nc.tensor.matmul(dps[:], esum[:, l * P:(l + 1) * P], ones[:],
                                 start=True, stop=True)
                dr = small.tile([P, 2], f32, tag="dr")
                nc.vector.tensor_copy(dr[:, 0:1], dps[:, 0:1])
                rr = dr[:, 1:2]
                nc.vector.reciprocal(rr[:], dr[:, 0:1])
                o = outp.tile([P, HD], f32, tag="o")
                nc.vector.tensor_scalar_mul(o[:, 0:512], avs[0][:], rr[:])
                nc.sync.dma_start(out.ap()[l * P:(l + 1) * P, 0:512],
                                  o[:, 0:512])
                nc.scalar.activation(o[:, 512:1024], avs[1][:], Act.Copy,
                                     scale=rr[:])
                nc.sync.dma_start(out.ap()[l * P:(l + 1) * P, 512:1024],
                                  o[:, 512:1024])

    nc.compile()
    return nc


def _build_general(use_pad: bool, use_vbias: bool):
    import concourse.bacc as bacc
    import concourse.mybir as mybir
    import concourse.tile as tile

    f32 = mybir.dt.float32
    f32r = mybir.dt.float32r
    bf16 = mybir.dt.bfloat16
    Act = mybir.ActivationFunctionType

    NCH = 4

    nc = bacc.Bacc("TRN2", num_swdge_queues=4, dynamic_dma_scratch_size=2048)

    qt = nc.dram_tensor("qt", [MD, SQL], f32r, kind="ExternalInput")
    kt = nc.dram_tensor("kt", [MD, S], f32r, kind="ExternalInput")
    vt = nc.dram_tensor("vt", [MD, S], f32r, kind="ExternalInput")
    wqt = nc.dram_tensor("wqt", [MD, HD], f32r, kind="ExternalInput")
    wkt = nc.dram_tensor("wkt", [MD, HD], f32r, kind="ExternalInput")
    wvt = nc.dram_tensor("wvt", [MD, HD], f32r, kind="ExternalInput")
    bq = nc.dram_tensor("bq", [HD], f32, kind="ExternalInput")
    bk = nc.dram_tensor("bk", [HD], f32, kind="ExternalInput")
    masks = nc.dram_tensor("masks", [4, P, 256], bf16, kind="ExternalInput")
    if use_pad:
        padm = nc.dram_tensor("padm", [P, NB], f32, kind="ExternalInput")
    if use_vbias:
        bv = nc.dram_tensor("bv", [HD], f32, kind="ExternalInput")
    out = nc.dram_tensor("out", [SQL, HD], f32, kind="ExternalOutput")

    with tile.TileContext(nc) as tc:
        with (
            tc.tile_pool(name="res", bufs=1) as res,
            tc.tile_pool(name="w", bufs=10) as wpool,
            tc.tile_pool(name="xin", bufs=4) as xin,
            tc.tile_pool(name="exp", bufs=16) as epool,
            tc.tile_pool(name="outp", bufs=1) as outp,
            tc.tile_pool(name="small", bufs=2) as small,
            tc.tile_pool(name="mm", bufs=5, space="PSUM") as mmp,
            tc.tile_pool(name="sc", bufs=2, space="PSUM") as scp,
            tc.tile_pool(name="dn", bufs=1, space="PSUM") as dnp,
        ):
            qh = res.tile([P, HB, SQL], f32r, tag="qh")
            kh = res.tile([P, HB, S], f32r, tag="kh")
            vh = res.tile([P, NB, HD], bf16, tag="vh")
            mt = res.tile([P, 4, 256], bf16, tag="mt")
            nc.scalar.dma_start(mt[:], masks.ap().rearrange("i p n -> p i n"))
            ones = res.tile([P, 2], bf16, tag="ones")
            nc.vector.memset(ones[:], 1.0)
            bias_t = res.tile([P, 2 * HB], f32, tag="bias")
            bqt = bias_t[:, 0:HB]
            nc.gpsimd.dma_start(bqt[:], bq.ap().rearrange("(hb p) -> p hb", p=P))
            bkt = bias_t[:, HB:2 * HB]
            nc.gpsimd.dma_start(bkt[:], bk.ap().rearrange("(hb p) -> p hb", p=P))
            if use_pad:
                pad_t = res.tile([P, NB], f32, tag="pad")
                nc.gpsimd.dma_start(pad_t[:], padm.ap())
            if use_vbias:
                ones_row = res.tile([1, P], f32r, tag="or")
                bvr = res.tile([1, HD], f32r, tag="bvr")
                nc.gpsimd.memset(ones_row[:].bitcast(f32), 1.0)
                nc.gpsimd.dma_start(bvr[:], bv.ap()[None, :])

            def load_w(dram, split=True):
                tiles = []
                for mc in range(MC):
                    t = wpool.tile([P, HD], f32r, tag="w", name=f"w{mc}")
                    weng = nc.scalar if (mc % 2 == 0 or not split) else nc.sync
                    weng.dma_start(t[:], dram.ap()[mc * P:(mc + 1) * P, :])
                    tiles.append(t)
                return tiles

            class XPair:
                def __init__(self, a, b):
                    self.a, self.b = a, b

                def __getitem__(self, key):
                    _, mc, cols = key
                    t = self.a if mc < 4 else self.b
                    return t[:, mc % 4, cols]

            def load_x(dram, c0):
                r = dram.ap().rearrange("(mc p) s -> p mc s", p=P)
                a = xin.tile([P, 4, 512], f32r, tag="x", name="xa")
                nc.sync.dma_start(a[:], r[:, 0:4, c0:c0 + 512])
                b = xin.tile([P, 4, 512], f32r, tag="x", name="xb")
                nc.sync.dma_start(b[:], r[:, 4:8, c0:c0 + 512])
                return XPair(a, b)

            # ---- Q projection: qh[:, hb, sq] (h on partitions) ----
            wq_t = load_w(wqt, split=False)
            for sqc in range(SQL // 512):
                xts = load_x(qt, sqc * 512)
                for hb in range(HB):
                    ps = mmp.tile([P, 512], f32, tag="mm")
                    for mc in range(MC):
                        nc.tensor.matmul(
                            ps[:], wq_t[mc][:, hb * P:(hb + 1) * P], xts[:, mc, :],
                            start=(mc == 0), stop=(mc == MC - 1))
                    nc.vector.tensor_scalar_add(
                        qh[:, hb, sqc * 512:(sqc + 1) * 512], ps[:],
                        bqt[:, hb:hb + 1])

            # ---- K projection: kh[:, hb, sk] ----
            wk_t = load_w(wkt)
            for skc in range(S // 512):
                xts = load_x(kt, skc * 512)
                for hb in range(HB):
                    ps = mmp.tile([P, 512], f32, tag="mm")
                    for mc in range(MC):
                        nc.tensor.matmul(
                            ps[:], wk_t[mc][:, hb * P:(hb + 1) * P], xts[:, mc, :],
                            start=(mc == 0), stop=(mc == MC - 1))
                    nc.vector.tensor_scalar_add(
                        kh[:, hb, skc * 512:(skc + 1) * 512], ps[:],
                        bkt[:, hb:hb + 1])

            # ---- V projection: vh[:, skb, h] (keys on partitions) ----
            wv_t = load_w(wvt)
            for skc in range(S // 512):
                xts = load_x(vt, skc * 512)
                for sbl in range(4):
                    skb = skc * 4 + sbl
                    for hc in range(2):
                        ps = mmp.tile([P, 512], f32, tag="mm")
                        for mc in range(MC):
                            nc.tensor.matmul(
                                ps[:], xts[:, mc, sbl * P:(sbl + 1) * P],
                                wv_t[mc][:, hc * 512:(hc + 1) * 512],
                                start=(mc == 0),
                                stop=(mc == MC - 1) and not use_vbias)
                        if use_vbias:
                            nc.tensor.matmul(
                                ps[:], ones_row[:],
                                bvr[:, hc * 512:(hc + 1) * 512],
                                start=False, stop=True)
                        nc.vector.tensor_copy(vh[:, skb, hc * 512:(hc + 1) * 512], ps[:])

            # ---- attention, chunk j = 256 queries, keys [0, (4j+4)*128) ----
            for j in range(NCH):
                E = 4 * j + 4
                sq0 = j * 256
                exps = []
                for kb in range(E):
                    sps = scp.tile([P, 256], f32, tag="s")
                    for hb in range(HB):
                        nc.tensor.matmul(
                            sps[:], kh[:, hb, kb * P:(kb + 1) * P],
                            qh[:, hb, sq0:sq0 + 256],
                            start=(hb == 0), stop=(hb == HB - 1))
                    ex = epool.tile([P, 256], bf16, tag="e")
                    nc.scalar.activation(ex[:], sps[:], Act.Exp, scale=1.0 / 32.0)
                    if kb >= 4 * j:
                        nc.vector.tensor_mul(ex[:], ex[:], mt[:, kb - 4 * j, :])
                    if use_pad:
                        nc.vector.tensor_scalar_mul(ex[:], ex[:], pad_t[:, kb:kb + 1])
                    exps.append(ex)

                for t in range(2):
                    dps = dnp.tile([P, 2], f32, tag="d")
                    avs = [mmp.tile([P, 512], f32, tag="mm", name=f"av{j}_{t}_{hc2}")
                           for hc2 in range(2)]
                    for kb in range(E):
                        lhs = exps[kb][:, t * P:(t + 1) * P]
                        for hc in range(2):
                            nc.tensor.matmul(
                                avs[hc][:], lhs, vh[:, kb, hc * 512:(hc + 1) * 512],
                                start=(kb == 0), stop=(kb == E - 1))
                        nc.tensor.matmul(
                            dps[:], lhs, ones[:],
                            start=(kb == 0), stop=(kb == E - 1))
                    dr = small.tile([P, 2], f32, tag="dr")
                    nc.vector.tensor_copy(dr[:, 0:1], dps[:, 0:1])
                    rr = dr[:, 1:2]
                    nc.vector.reciprocal(rr[:], dr[:, 0:1])
                    o = outp.tile([P, HD], f32, tag="o")
                    lb = 2 * j + t
                    nc.vector.tensor_scalar_mul(o[:, 0:512], avs[0][:], rr[:])
                    nc.sync.dma_start(out.ap()[lb * P:(lb + 1) * P, 0:512],
                                      o[:, 0:512])
                    nc.scalar.activation(o[:, 512:1024], avs[1][:], Act.Copy,
                                         scale=rr[:])
                    nc.sync.dma_start(out.ap()[lb * P:(lb + 1) * P, 512:1024],
                                      o[:, 512:1024])

    nc.compile()
    return nc


@contextlib.contextmanager
def _ntff_profile(output_dir, device_ids):
    """NTFF capture via the axon PJRT .so C ABI (test-only; needs
    ATTN_PROF_DIR set)."""
    import ctypes
    import jax

    lib = ctypes.CDLL("/opt/axon/libaxon_pjrt.so")
    lib.axon_start_nrt_profile.argtypes = [
        ctypes.POINTER(ctypes.c_int64), ctypes.c_size_t]
    lib.axon_start_nrt_profile.restype = ctypes.c_int64
    lib.axon_stop_nrt_profile.argtypes = [ctypes.c_char_p]
    lib.axon_stop_nrt_profile.restype = ctypes.c_int64
    jax.devices()
    ids = (ctypes.c_int64 * len(device_ids))(*device_ids)
    rc = lib.axon_start_nrt_profile(ids, len(device_ids))
    if rc != 0:
        raise RuntimeError(f"axon_start_nrt_profile rc={rc}")
    try:
        yield
    finally:
        n = lib.axon_stop_nrt_profile(str(output_dir).encode())
        print(f"profile: {n} file(s) written to {output_dir}")


def kernel(q, k, v, attention_mask, Wq_w, Wq_b, Wk_w, Wk_b, Wv_w, Wv_b):
    import ml_dtypes
    from concourse.bass_utils import run_bass_kernel_spmd

    bf16 = ml_dtypes.bfloat16

    q = np.asarray(q, dtype=np.float32)
    k = np.asarray(k, dtype=np.float32)
    v = np.asarray(v, dtype=np.float32)
    am = np.asarray(attention_mask)

    use_pad = not bool((am == 1).all())
    use_vbias = bool(np.any(np.asarray(Wv_b) != 0))
    use_qkbias = bool(np.any(np.asarray(Wq_b) != 0) or np.any(np.asarray(Wk_b) != 0))

    perms = []
    for c in range(2):
        perm = np.concatenate([np.arange(P) + (2 * l + c) * P for l in range(8)])
        perms.append(perm)

    if use_qkbias:
        nc = _build_general(use_pad, use_vbias)
        wqt = np.ascontiguousarray(np.asarray(Wq_w, np.float32).T)
        wkt = np.ascontiguousarray(np.asarray(Wk_w, np.float32).T)
        wvt = np.ascontiguousarray(np.asarray(Wv_w, np.float32).T)
        bq = np.ascontiguousarray(np.asarray(Wq_b, np.float32))
        bk = np.ascontiguousarray(np.asarray(Wk_b, np.float32))
        bv = np.ascontiguousarray(np.asarray(Wv_b, np.float32))

        # general path: chunk j = global blocks {4j+c, 4j+c+2}; causal masks
        # for the 4 tail key-blocks of each chunk, per half c.
        gperms = []
        for c in range(2):
            gperm = np.concatenate([
                np.arange(P) + (4 * j + c + 2 * t) * P
                for j in range(4) for t in range(2)
            ])
            gperms.append(gperm)
        mask_c = []
        a = np.arange(P)[:, None]
        col = np.arange(256)[None, :]
        for c in range(2):
            t = col // P
            b_ = col % P
            m = np.stack([
                (128 * i + a <= 128 * (c + 2 * t) + b_) for i in range(4)
            ]).astype(np.float32)
            mask_c.append(m.astype(bf16))

        kT = [np.ascontiguousarray(k[b].T) for b in range(B)]
        vT = [np.ascontiguousarray(v[b].T) for b in range(B)]
        in_maps = []
        for cid in range(N_CORES):
            b, c = cid // 2, cid % 2
            qT = np.ascontiguousarray(q[b].T[:, gperms[c]])
            m = dict(qt=qT, kt=kT[b], vt=vT[b], wqt=wqt, wkt=wkt, wvt=wvt,
                     bq=bq, bk=bk, masks=mask_c[c])
            if use_pad:
                padv = am[b].astype(np.float32)
                m["padm"] = np.ascontiguousarray(padv.reshape(NB, P).T)
            if use_vbias:
                m["bv"] = bv
            in_maps.append(m)
        out_perms = gperms
    else:
        nc = _build_fast(use_pad, use_vbias)

        def pmaj(x):
            # [MD, N] -> partition-major [P, MC, N] (SBUF layout, so each
            # per-partition DMA segment is one contiguous run)
            x = np.asarray(x, np.float32).astype(bf16)
            return np.ascontiguousarray(
                x.reshape(MC, P, x.shape[1]).transpose(1, 0, 2))

        A = (np.asarray(Wq_w, np.float64).T @ np.asarray(Wk_w, np.float64))
        A = pmaj(A.astype(np.float32))
        wvt = pmaj(np.asarray(Wv_w, np.float32).T)
        bv = np.ascontiguousarray(np.asarray(Wv_b, np.float32).astype(bf16))

        # boundary masks per (c, slot i), first 128 cols of the slot only:
        # tri (diagonal q-block), zero (invalid), or ones.
        a_i = np.arange(P)[:, None]
        b_i = np.arange(P)[None, :]
        tri = (a_i <= b_i).astype(np.float32)
        onesm = np.ones((P, P), np.float32)
        zerom = np.zeros((P, P), np.float32)
        mask_c = []
        for c in range(2):
            slots = []
            for i in range(NB):
                if c == 0:
                    first = tri if i % 2 == 0 else zerom
                else:
                    first = tri if i % 2 == 1 else onesm
                slots.append(first)
            m = np.stack(slots)  # [NB, P, P]
            mask_c.append(np.ascontiguousarray(
                m.transpose(1, 0, 2)).astype(bf16))

        kT = [pmaj(k[b].T) for b in range(B)]
        vT = [pmaj(v[b].T) for b in range(B)]
        in_maps = []
        for cid in range(N_CORES):
            b, c = cid // 2, cid % 2
            qT = pmaj(q[b].T[:, perms[c]])
            m = dict(qt=qT, kt=kT[b], vt=vT[b], at=A, wvt=wvt,
                     masks=mask_c[c])
            if use_pad:
                padv = am[b].astype(np.float32)
                m["padm"] = np.ascontiguousarray(padv.reshape(NB, P).T)
            if use_vbias:
                m["bv"] = bv
            in_maps.append(m)
        out_perms = perms

    prof_dir = os.environ.get("ATTN_PROF_DIR")
    if prof_dir:
        with _ntff_profile(prof_dir, [0]):
            res = run_bass_kernel_spmd(nc, in_maps, list(range(N_CORES)))
    else:
        res = run_bass_kernel_spmd(nc, in_maps, list(range(N_CORES)))

    out = np.empty((B, S, HD), np.float32)
    for cid in range(N_CORES):
        b, c = cid // 2, cid % 2
        oc = res.results[cid]["out"]
        out[b, out_perms[c], :] = oc
    return out
